# revision 5
# baseline (speedup 1.0000x reference)
"""Trainium2 Bass kernel for nn_AffineAdapter (Gaussian blur + affine grid_sample).

The reference pipeline (separable 8-tap Gaussian blur -> bilinear grid_sample on a
25x25 grid, align_corners=True, zero padding) is linear in x and separable per
axis, so each (b, c) image reduces to   out = Ay @ X @ Ax^T   with Ay, Ax of
shape (25, 512) combining blur taps and bilinear weights.  Output sample row p
only reads the 9 input rows [y0(p)-3, y0(p)+6) ("band"), so the device gathers
just 25 bands x 9 rows = 225 rows per image, cropped to the needed column
window, via banded DMA descriptors.

Distribution: pure data parallel over B*C = 128 images -> 16 images per core
on 8 NeuronCores.  Per core on-device:
  stage 1:  tmpT[w, p] (per img) = sum_h Xg[h, w] * Ayg[p, h]   (X as stationary
            operand so the surviving w axis lands on PSUM partitions)
  stage 2:  out[q, (img, p)] = sum_w Ax[q, w] * tmpT[w, (img, p)]  (one matmul
            per w-chunk for ALL images)
Host computes Ay/Ax from the runtime log_sigma/log_scale inputs, shards x,
gathers (25, 16, 25) per-core outputs and transposes back to (B, C, 25, 25).
"""

import sys

if "/opt/trn_rl_repo" not in sys.path:
    sys.path.insert(0, "/opt/trn_rl_repo")

import numpy as np

GRID = 25
K = 7
KH = K // 2          # conv padding = 3
NTAPS = K + 1        # 8 taps (torch arange quirk)
BAND = NTAPS + 1     # 9 rows per output sample row
NG = GRID * BAND     # 225 gathered rows per image
NCH = -(-NG // 128)  # 2 gathered-row chunks
H = W = 512
B, C = 16, 8
N_CORES = 8
NIMG = (B * C) // N_CORES  # images per core

# matmul operand dtype: "f32" (exact, 2 HW passes) or "f32r" (tf32-like, 1 pass)
MM_DTYPE = "f32"


def _softplus(v):
    v = np.asarray(v)
    return np.log1p(np.exp(-np.abs(v))) + np.maximum(v, 0.0)


def _axis_weights(lin, g, scale_ax, n_in):
    """(GRID, n_in) float64 matrix combining blur taps + bilinear sample weights,
    plus per-sample band starts r0 with guaranteed support A[p] in [r0[p], r0[p]+9)."""
    nb = n_in - 1  # blurred length (conv with K+1 taps, pad K//2 shrinks by 1)
    coord = ((lin * np.float32(scale_ax) + np.float32(1.0))
             * np.float32(0.5) * np.float32(nb - 1)).astype(np.float32)
    c0 = np.floor(coord)
    w1 = (coord - c0).astype(np.float64)
    w0 = 1.0 - w1
    A = np.zeros((GRID, n_in), np.float64)
    g64 = g.astype(np.float64)
    r0 = np.zeros(GRID, np.int64)
    for p in range(GRID):
        r0[p] = int(min(max(c0[p] - KH, 0), n_in - BAND))
        for a, wgt in ((0, w0[p]), (1, w1[p])):
            cc = float(c0[p]) + a
            if not (0.0 <= cc <= nb - 1):
                continue  # zero padding_mode: out-of-range corner contributes 0
            ci = int(min(max(cc, 0.0), nb - 1))
            # blurred[ci] = sum_i g[i] * x[ci + i - KH]
            for i in range(NTAPS):
                src = ci + i - KH
                if 0 <= src < n_in:
                    A[p, src] += wgt * g64[i]
    return A, r0


def _build_weights(log_sigma, log_scale):
    # scalar chain in fp32 to mirror the reference
    scale = _softplus(np.asarray(log_scale, np.float32)).astype(np.float32)
    s_min = np.float32(scale.min())
    sigma_min = np.float32(0.0) if s_min >= 1.0 else np.float32(0.44) * (
        np.float32(1.0) / s_min - np.float32(1.0))
    sigma = np.float32(np.sqrt(sigma_min ** 2
                               + _softplus(np.asarray(log_sigma, np.float32)) ** 2))
    taps = np.arange(-(KH + 1), KH + 1, dtype=np.float32)
    g = np.exp(-0.5 * (taps / sigma) ** 2)
    g = g / g.sum()

    lin = np.linspace(-1.0, 1.0, GRID).astype(np.float32)
    Ay, ry = _axis_weights(lin, g, scale[1], H)  # rows scaled by scale[1] (y)
    Ax, _ = _axis_weights(lin, g, scale[0], W)   # cols scaled by scale[0] (x)
    return Ay.astype(np.float32), Ax.astype(np.float32), ry


def _col_window(Amat, n_in):
    """Smallest [start, start + ncw*128) window covering A's nonzero columns."""
    used = np.nonzero(Amat.any(axis=0))[0]
    if len(used) == 0:
        return 0, 1
    lo, hi = int(used[0]), int(used[-1]) + 1
    ncw = min(4, max(1, -(-(hi - lo) // 128)))
    start = max(0, min(lo, n_in - ncw * 128))
    if hi > start + ncw * 128:
        ncw, start = 4, 0
    return start, ncw


_PROGRAM_CACHE = {}


def _build_program(ncw, w0, ry):
    import concourse.tile as tile
    from concourse import bacc, mybir
    from concourse.bass import ts

    f32 = mybir.dt.float32
    mmdt = {"f32": mybir.dt.float32, "f32r": mybir.dt.float32r}[MM_DTYPE]
    wsz = ncw * 128

    nc = bacc.Bacc("TRN2", target_bir_lowering=False, debug=False,
                   num_devices=N_CORES)
    xs = nc.dram_tensor("xs", [NIMG, H, W], mmdt, kind="ExternalInput")
    ayt = nc.dram_tensor("ayt", [NCH, 128, GRID], mmdt, kind="ExternalInput")
    axt = nc.dram_tensor("axt", [ncw, 128, GRID], mmdt, kind="ExternalInput")
    out = nc.dram_tensor("out", [GRID, NIMG, GRID], f32, kind="ExternalOutput")

    with tile.TileContext(nc) as tc:
        with (
            tc.tile_pool(name="const", bufs=1) as const_pool,
            tc.tile_pool(name="ps1", bufs=4, space="PSUM") as psum1,
            tc.tile_pool(name="ps2", bufs=1, space="PSUM") as psum2,
        ):
            aytile = const_pool.tile([128, NCH, GRID], mmdt)
            for c in range(NCH):
                nc.sync.dma_start(out=aytile[:, c, :], in_=ayt[c])
            axtile = const_pool.tile([128, ncw, GRID], mmdt)
            for c in range(ncw):
                nc.sync.dma_start(out=axtile[:, c, :], in_=axt[c])

            # gathered-row tiles: (row-chunk) x (partition=gathered row,
            # free=(img, w)).  Banded DMA: one transfer per (band, chunk piece)
            # covering all images.
            xg = [const_pool.tile([128, NIMG, wsz], mmdt, name=f"xg{c}")
                  for c in range(NCH)]
            for p in range(GRID):
                pos = p * BAND
                left = BAND
                while left > 0:
                    chunk, off = pos // 128, pos % 128
                    n = min(left, 128 - off)
                    src = xs[:, int(ry[p]) + (BAND - left):int(ry[p]) + (BAND - left) + n,
                             w0:w0 + wsz].rearrange("i r w -> r i w")
                    nc.sync.dma_start(out=xg[chunk][off:off + n, :, :], in_=src)
                    pos += n
                    left -= n

            # stage 1: per (img, w-chunk): tmpT[w, p] accumulated over row chunks
            tm = const_pool.tile([128, ncw, NIMG, GRID], mmdt)
            nrows = [128, NG - 128]  # rows used per gathered chunk
            for img in range(NIMG):
                for cw in range(ncw):
                    ps = psum1.tile([128, GRID], f32)
                    for c in range(NCH):
                        r = nrows[c]
                        nc.tensor.matmul(
                            ps[:],
                            xg[c][:r, img, ts(cw, 128)],   # lhsT (K=h, M=w)
                            aytile[:r, c, :],              # rhs  (K=h, N=p)
                            start=(c == 0),
                            stop=(c == NCH - 1),
                        )
                    nc.vector.tensor_copy(tm[:, cw, img, :], ps[:])

            # stage 2: one matmul per w-chunk for all images
            po = psum2.tile([GRID, NIMG, GRID], f32)
            for cw in range(ncw):
                nc.tensor.matmul(
                    po[:],
                    axtile[:, cw, :],                      # lhsT (K=w, M=q)
                    tm[:, cw, :, :],                       # rhs  (K=w, N=(img,p))
                    start=(cw == 0),
                    stop=(cw == ncw - 1),
                )
            outst = const_pool.tile([GRID, NIMG, GRID], f32)
            nc.vector.tensor_copy(outst[:], po[:])
            nc.sync.dma_start(out=out[:], in_=outst[:])

    nc.compile()
    return nc


def _get_program(ncw, w0, ry):
    key = (MM_DTYPE, ncw, w0, tuple(int(r) for r in ry))
    if key not in _PROGRAM_CACHE:
        _PROGRAM_CACHE[key] = _build_program(ncw, w0, ry)
    return _PROGRAM_CACHE[key]


def _prepare(log_sigma, log_scale):
    Ay, Ax, ry = _build_weights(log_sigma, log_scale)
    w0, ncw = _col_window(Ax, W)

    # gathered Ay chunks with per-band masking: gathered row k (band b = k//9,
    # source row ry[b] + k%9) contributes only to output sample p == b.
    ayt = np.zeros((NCH, 128, GRID), np.float32)
    for k in range(NG):
        b, j = divmod(k, BAND)
        val = Ay[b, ry[b] + j]
        ayt[k // 128, k % 128, b] = val
    # sanity: each sample's support must lie inside its band
    for p in range(GRID):
        sup = np.nonzero(Ay[p])[0]
        if len(sup) and not (ry[p] <= sup[0] and sup[-1] < ry[p] + BAND):
            raise AssertionError("band does not cover sample support")

    def colchunks(A, start, nchk):
        pad = np.zeros((GRID, max(0, start + nchk * 128 - A.shape[1])), np.float32)
        Aw = np.concatenate([A, pad], axis=1)[:, start:start + nchk * 128]
        return np.ascontiguousarray(Aw.T.reshape(nchk, 128, GRID).astype(np.float32))

    axt = colchunks(Ax, w0, ncw)
    return ayt, axt, ry, w0, ncw


def kernel(x, log_sigma, log_scale):
    from concourse.bass_utils import run_bass_kernel_spmd

    x = np.ascontiguousarray(np.asarray(x, np.float32))
    assert x.shape == (B, C, H, W), x.shape

    ayt, axt, ry, w0, ncw = _prepare(log_sigma, log_scale)
    nc = _get_program(ncw, w0, ry)

    xf = x.reshape(B * C, H, W)
    in_maps = [
        {"xs": xf[i * NIMG:(i + 1) * NIMG], "ayt": ayt, "axt": axt}
        for i in range(N_CORES)
    ]
    res = run_bass_kernel_spmd(nc, in_maps, core_ids=list(range(N_CORES)))

    out = np.empty((B * C, GRID, GRID), np.float32)
    for i in range(N_CORES):
        # per-core output is (GRID, NIMG, GRID) = (q, img, p)
        out[i * NIMG:(i + 1) * NIMG] = res.results[i]["out"].transpose(1, 2, 0)
    return out.reshape(B, C, GRID, GRID)


# revision 9
# speedup vs baseline: 1.6691x; 1.6691x over previous
"""Trainium2 Bass kernel for nn_AffineAdapter (Gaussian blur + affine grid_sample).

The reference pipeline (separable 8-tap Gaussian blur -> bilinear grid_sample on a
25x25 grid, align_corners=True, zero padding) is linear in x and separable per
axis, so each (b, c) image reduces to   out = Ay @ X @ Ax^T   with Ay, Ax of
shape (25, 512) combining blur taps and bilinear weights.  Output sample row p
only reads the 9 input rows [y0(p)-3, y0(p)+6) ("band"), so the device gathers
just 25 bands x 9 rows = 225 rows per image, cropped to the needed column
window.  Gathered rows are interleaved across SBUF partitions (row j of band p
-> partition (j % 5) * 25 + p) so each banded DMA engages many SBUF ports, and
band transfers alternate between the two HWDGE rings (sync / scalar).

Distribution: pure data parallel over B*C = 128 images -> 16 images per core
on 8 NeuronCores.  Per core on-device:
  stage 1:  tmpT[w, p] (per img) = sum_h Xg[h, w] * Ayg[p, h]   (X as the
            stationary operand so the surviving w axis lands on PSUM partitions)
  stage 2:  out[q, (img, p)] = sum_w Ax[q, w] * tmpT[w, (img, p)]  (one matmul
            per w-chunk for ALL images)
Host computes Ay/Ax from the runtime log_sigma/log_scale inputs, shards x,
gathers (25, 16, 25) per-core outputs and transposes back to (B, C, 25, 25).

MM_DTYPE selects the stage-1 numeric path:
  "f32"    exact fp32 (2 HW passes per matmul)
  "f32r"   tf32-like single pass (~1e-4 rel error)
  "bf16x2" X and Ay split into bf16 hi+lo on host; 2 bf16 passes reproduce
           fp32-grade accuracy with fast FWL weight loads
"""

import sys

if "/opt/trn_rl_repo" not in sys.path:
    sys.path.insert(0, "/opt/trn_rl_repo")

import numpy as np

GRID = 25
K = 7
KH = K // 2          # conv padding = 3
NTAPS = K + 1        # 8 taps (torch arange quirk)
BAND = NTAPS + 1     # 9 rows per output sample row
J0 = 5               # band rows 0..4 -> chunk 0 (partition j*25 + p)
J1 = BAND - J0       # band rows 5..8 -> chunk 1 (partition (j-5)*25 + p)
NR = (J0 * GRID, J1 * GRID)   # rows used per gathered chunk: (125, 100)
H = W = 512
B, C = 16, 8
N_CORES = 8
NIMG = (B * C) // N_CORES  # images per core

MM_DTYPE = "bf16x2"  # "f32" | "f32r" | "bf16x2"


def _softplus(v):
    v = np.asarray(v)
    return np.log1p(np.exp(-np.abs(v))) + np.maximum(v, 0.0)


def _axis_weights(lin, g, scale_ax, n_in):
    """(GRID, n_in) float64 weight matrix + per-sample band starts r0 such that
    the support of row p lies in [r0[p], r0[p] + BAND)."""
    nb = n_in - 1  # blurred length (conv with K+1 taps, pad K//2 shrinks by 1)
    coord = ((lin * np.float32(scale_ax) + np.float32(1.0))
             * np.float32(0.5) * np.float32(nb - 1)).astype(np.float32)
    c0 = np.floor(coord)
    w1 = (coord - c0).astype(np.float64)
    w0 = 1.0 - w1
    A = np.zeros((GRID, n_in), np.float64)
    g64 = g.astype(np.float64)
    r0 = np.zeros(GRID, np.int64)
    for p in range(GRID):
        r0[p] = int(min(max(c0[p] - KH, 0), n_in - BAND))
        for a, wgt in ((0, w0[p]), (1, w1[p])):
            cc = float(c0[p]) + a
            if not (0.0 <= cc <= nb - 1):
                continue  # zero padding_mode: out-of-range corner contributes 0
            ci = int(min(max(cc, 0.0), nb - 1))
            # blurred[ci] = sum_i g[i] * x[ci + i - KH]
            for i in range(NTAPS):
                src = ci + i - KH
                if 0 <= src < n_in:
                    A[p, src] += wgt * g64[i]
    return A, r0


def _build_weights(log_sigma, log_scale):
    # scalar chain in fp32 to mirror the reference
    scale = _softplus(np.asarray(log_scale, np.float32)).astype(np.float32)
    s_min = np.float32(scale.min())
    sigma_min = np.float32(0.0) if s_min >= 1.0 else np.float32(0.44) * (
        np.float32(1.0) / s_min - np.float32(1.0))
    sigma = np.float32(np.sqrt(sigma_min ** 2
                               + _softplus(np.asarray(log_sigma, np.float32)) ** 2))
    taps = np.arange(-(KH + 1), KH + 1, dtype=np.float32)
    g = np.exp(-0.5 * (taps / sigma) ** 2)
    g = g / g.sum()

    lin = np.linspace(-1.0, 1.0, GRID).astype(np.float32)
    Ay, ry = _axis_weights(lin, g, scale[1], H)  # rows scaled by scale[1] (y)
    Ax, _ = _axis_weights(lin, g, scale[0], W)   # cols scaled by scale[0] (x)
    return Ay, Ax, ry


def _col_window(Amat, n_in):
    """Smallest [start, start + ncw*128) window covering A's nonzero columns."""
    used = np.nonzero(Amat.any(axis=0))[0]
    if len(used) == 0:
        return 0, 1
    lo, hi = int(used[0]), int(used[-1]) + 1
    ncw = min(4, max(1, -(-(hi - lo) // 128)))
    start = max(0, min(lo, n_in - ncw * 128))
    if hi > start + ncw * 128:
        ncw, start = 4, 0
    return start, ncw


def _bf16_split(a32):
    import ml_dtypes
    hi = a32.astype(ml_dtypes.bfloat16)
    lo = (a32 - hi.astype(np.float32)).astype(ml_dtypes.bfloat16)
    return hi, lo


_PROGRAM_CACHE = {}


def _build_program(ncw, w0, ry):
    import concourse.tile as tile
    from concourse import bacc, mybir
    from concourse.bass import ts

    f32 = mybir.dt.float32
    bf16 = mybir.dt.bfloat16
    wsz = ncw * 128
    split = MM_DTYPE == "bf16x2"
    mmdt = {"f32": f32, "f32r": mybir.dt.float32r, "bf16x2": bf16}[MM_DTYPE]
    s2dt = f32 if split else mmdt     # stage-2 operand dtype
    nay = 2 * GRID if split else GRID  # stage-1 rhs width ([Ah|Al] when split)

    nc = bacc.Bacc("TRN2", target_bir_lowering=False, debug=False,
                   num_devices=N_CORES)
    if split:
        xs = nc.dram_tensor("xs", [NIMG, H, 2, wsz], bf16, kind="ExternalInput")
    else:
        xs = nc.dram_tensor("xs", [NIMG, H, W], mmdt, kind="ExternalInput")
    ayt = nc.dram_tensor("ayt", [2, 128, nay], mmdt, kind="ExternalInput")
    axt = nc.dram_tensor("axt", [ncw, 128, GRID], s2dt, kind="ExternalInput")
    out = nc.dram_tensor("out", [GRID, NIMG, GRID], f32, kind="ExternalOutput")

    with tile.TileContext(nc) as tc:
        with (
            tc.tile_pool(name="const", bufs=1) as const_pool,
            tc.tile_pool(name="ps1", bufs=6, space="PSUM") as psum1,
            tc.tile_pool(name="ps2", bufs=1, space="PSUM") as psum2,
        ):
            aytile = const_pool.tile([128, 2, nay], mmdt)
            for c in range(2):
                nc.sync.dma_start(out=aytile[:, c, :], in_=ayt[c])
            axtile = const_pool.tile([128, ncw, GRID], s2dt)
            for c in range(ncw):
                nc.sync.dma_start(out=axtile[:, c, :], in_=axt[c])

            # gathered-row tiles; row j of band p -> chunk j//J0,
            # partition (j % J0) * GRID + p  (interleave spreads SBUF ports)
            if split:
                xg = [const_pool.tile([128, NIMG, 2, wsz], mmdt, name=f"xg{c}")
                      for c in range(2)]
            else:
                xg = [const_pool.tile([128, NIMG, wsz], mmdt, name=f"xg{c}")
                      for c in range(2)]
            for p in range(GRID):
                eng = nc.sync if (p % 2 == 0) else nc.scalar
                for c, (jlo, jn) in enumerate(((0, J0), (J0, J1))):
                    r = int(ry[p]) + jlo
                    pend = p + (jn - 1) * GRID + 1
                    if split:
                        src = xs[:, r:r + jn, :, :].rearrange(
                            "i r t w -> r i (t w)")
                        dst = xg[c][p:pend:GRID, :, :, :].rearrange(
                            "j i t w -> j i (t w)")
                    else:
                        src = xs[:, r:r + jn, w0:w0 + wsz].rearrange(
                            "i r w -> r i w")
                        dst = xg[c][p:pend:GRID, :, :]
                    eng.dma_start(out=dst, in_=src)

            # stage 1: per (img, w-chunk): tmpT[w, :] accumulated over row
            # chunks (and over hi/lo passes when split)
            tm = const_pool.tile([128, ncw, NIMG, GRID], s2dt)
            for img in range(NIMG):
                for cw in range(ncw):
                    if split:
                        # psum cols interleaved (p, t): t=0 hi-weight prods,
                        # t=1 lo; evacuated with a single X-axis reduce
                        ps = psum1.tile([128, GRID, 2], f32)
                        for c in range(2):
                            r = NR[c]
                            for t in range(2):
                                nc.tensor.matmul(
                                    ps[:],
                                    xg[c][:r, img, t, ts(cw, 128)],
                                    aytile[:r, c, :],
                                    start=(c == 0 and t == 0),
                                    stop=(c == 1 and t == 1),
                                )
                        nc.vector.tensor_reduce(
                            tm[:, cw, img, :], ps[:],
                            axis=mybir.AxisListType.X, op=mybir.AluOpType.add)
                    else:
                        ps = psum1.tile([128, GRID], f32)
                        for c in range(2):
                            r = NR[c]
                            nc.tensor.matmul(
                                ps[:],
                                xg[c][:r, img, ts(cw, 128)],   # lhsT (K=h, M=w)
                                aytile[:r, c, :],              # rhs  (K=h, N=p)
                                start=(c == 0),
                                stop=(c == 1),
                            )
                        nc.vector.tensor_copy(tm[:, cw, img, :], ps[:])

            # stage 2: one matmul per w-chunk for all images
            po = psum2.tile([GRID, NIMG, GRID], f32)
            for cw in range(ncw):
                nc.tensor.matmul(
                    po[:],
                    axtile[:, cw, :],                      # lhsT (K=w, M=q)
                    tm[:, cw, :, :],                       # rhs  (K=w, N=(img,p))
                    start=(cw == 0),
                    stop=(cw == ncw - 1),
                )
            outst = const_pool.tile([GRID, NIMG, GRID], f32)
            nc.vector.tensor_copy(outst[:], po[:])
            nc.sync.dma_start(out=out[:], in_=outst[:])

    nc.compile()
    return nc


def _get_program(ncw, w0, ry):
    key = (MM_DTYPE, ncw, w0, tuple(int(r) for r in ry))
    if key not in _PROGRAM_CACHE:
        _PROGRAM_CACHE[key] = _build_program(ncw, w0, ry)
    return _PROGRAM_CACHE[key]


def _gather_ay(Ay, ry, split):
    """Per-chunk stage-1 rhs: gathered row k' = (j % J0)*GRID + p holds
    Ay[p, ry[p] + j], masked so it only feeds output sample p."""
    nay = 2 * GRID if split else GRID
    ayt64 = np.zeros((2, 128, nay), np.float64)
    for p in range(GRID):
        for j in range(BAND):
            c, jj = (0, j) if j < J0 else (1, j - J0)
            ayt64[c, jj * GRID + p, p] = Ay[p, int(ry[p]) + j]
    for p in range(GRID):
        sup = np.nonzero(Ay[p])[0]
        if len(sup) and not (ry[p] <= sup[0] and sup[-1] < ry[p] + BAND):
            raise AssertionError("band does not cover sample support")
    if not split:
        return ayt64.astype(np.float32)
    hi, lo = _bf16_split(ayt64[:, :, :GRID].astype(np.float32))
    outw = np.zeros((2, 128, nay), hi.dtype)
    outw[:, :, 0::2] = hi   # interleave (p, t): even cols hi, odd cols lo
    outw[:, :, 1::2] = lo
    return outw


def _prepare(log_sigma, log_scale):
    Ay, Ax, ry = _build_weights(log_sigma, log_scale)
    w0, ncw = _col_window(Ax, W)
    split = MM_DTYPE == "bf16x2"

    ayt = _gather_ay(Ay, ry, split)

    def colchunks(A, start, nchk):
        pad = np.zeros((GRID, max(0, start + nchk * 128 - A.shape[1])))
        Aw = np.concatenate([A, pad], axis=1)[:, start:start + nchk * 128]
        return np.ascontiguousarray(Aw.T.reshape(nchk, 128, GRID)).astype(np.float32)

    axt = colchunks(Ax, w0, ncw)
    return ayt, axt, ry, w0, ncw


def kernel(x, log_sigma, log_scale):
    from concourse.bass_utils import run_bass_kernel_spmd

    x = np.ascontiguousarray(np.asarray(x, np.float32))
    assert x.shape == (B, C, H, W), x.shape

    ayt, axt, ry, w0, ncw = _prepare(log_sigma, log_scale)
    nc = _get_program(ncw, w0, ry)

    xf = x.reshape(B * C, H, W)
    if MM_DTYPE == "bf16x2":
        wsz = ncw * 128
        crop = np.zeros((B * C, H, wsz), np.float32)
        lo_w, hi_w = w0, min(W, w0 + wsz)
        crop[:, :, :hi_w - lo_w] = xf[:, :, lo_w:hi_w]
        hi, lo = _bf16_split(crop)
        xp = np.ascontiguousarray(
            np.stack([hi, lo], axis=2))          # (BC, H, 2, wsz) bf16
        shards = [xp[i * NIMG:(i + 1) * NIMG] for i in range(N_CORES)]
    else:
        shards = [xf[i * NIMG:(i + 1) * NIMG] for i in range(N_CORES)]

    in_maps = [{"xs": shards[i], "ayt": ayt, "axt": axt} for i in range(N_CORES)]
    res = run_bass_kernel_spmd(nc, in_maps, core_ids=list(range(N_CORES)))

    out = np.empty((B * C, GRID, GRID), np.float32)
    for i in range(N_CORES):
        # per-core output is (GRID, NIMG, GRID) = (q, img, p)
        out[i * NIMG:(i + 1) * NIMG] = res.results[i]["out"].transpose(1, 2, 0)
    return out.reshape(B, C, GRID, GRID)


# revision 10
# speedup vs baseline: 2.6633x; 1.5956x over previous
"""Trainium2 Bass kernel for nn_AffineAdapter (Gaussian blur + affine grid_sample).

The reference pipeline (separable 8-tap Gaussian blur -> bilinear grid_sample on a
25x25 grid, align_corners=True, zero padding) is linear in x and separable per
axis, so each (b, c) image reduces to   out = Ay @ X @ Ax^T   with Ay, Ax of
shape (25, 512) combining blur taps and bilinear weights.  Output sample row p
only reads the 9 input rows [y0(p)-3, y0(p)+6) ("band") and similarly only a
~362-column window is ever touched, so only 25*9 = 225 rows x 384 cols of each
512x512 image carry information.

Distribution/layout: pure data parallel over B*C = 128 images -> 16 images per
core on 8 NeuronCores.  While sharding, the host packs each image's 225 banded
rows (cropped to the column window) into a dense block, splitting values into
bf16 hi + lo halves (x = hi + lo exactly; products recover full fp32 accuracy
while TensorE runs at bf16 speed with fast weight loads).  Each core's shard is
(16, 256, 2, 384) bf16; the device reads it with one 128-partition DMA per
image.

Per core on-device:
  stage 1:  tmpT[w, p] (per img) = sum_k Xg[k, w] * Ayg[k, p] over the 225
            gathered rows (2 chunks x hi/lo passes; X is the stationary
            operand so the surviving w axis lands on PSUM partitions)
  stage 2:  out[q, (img, p)] = sum_w Ax[q, w] * tmpT[w, (img, p)]  (one fp32
            matmul per w-chunk for ALL images)
Host computes Ay/Ax from the runtime log_sigma/log_scale inputs and transposes
the gathered (25, 16, 25) per-core outputs back to (B, C, 25, 25).
"""

import sys

if "/opt/trn_rl_repo" not in sys.path:
    sys.path.insert(0, "/opt/trn_rl_repo")

import numpy as np

GRID = 25
K = 7
KH = K // 2          # conv padding = 3
NTAPS = K + 1        # 8 taps (torch arange quirk)
BAND = NTAPS + 1     # 9 rows per output sample row
NG = GRID * BAND     # 225 gathered rows per image
NGP = 256            # padded to 2 x 128 partitions (rows 225.. are zero)
H = W = 512
B, C = 16, 8
N_CORES = 8
NIMG = (B * C) // N_CORES  # images per core


def _softplus(v):
    v = np.asarray(v)
    return np.log1p(np.exp(-np.abs(v))) + np.maximum(v, 0.0)


def _axis_weights(lin, g, scale_ax, n_in):
    """(GRID, n_in) float64 weight matrix + per-sample band starts r0 such that
    the support of row p lies in [r0[p], r0[p] + BAND)."""
    nb = n_in - 1  # blurred length (conv with K+1 taps, pad K//2 shrinks by 1)
    coord = ((lin * np.float32(scale_ax) + np.float32(1.0))
             * np.float32(0.5) * np.float32(nb - 1)).astype(np.float32)
    c0 = np.floor(coord)
    w1 = (coord - c0).astype(np.float64)
    w0 = 1.0 - w1
    A = np.zeros((GRID, n_in), np.float64)
    g64 = g.astype(np.float64)
    r0 = np.zeros(GRID, np.int64)
    for p in range(GRID):
        r0[p] = int(min(max(c0[p] - KH, 0), n_in - BAND))
        for a, wgt in ((0, w0[p]), (1, w1[p])):
            cc = float(c0[p]) + a
            if not (0.0 <= cc <= nb - 1):
                continue  # zero padding_mode: out-of-range corner contributes 0
            ci = int(min(max(cc, 0.0), nb - 1))
            # blurred[ci] = sum_i g[i] * x[ci + i - KH]
            for i in range(NTAPS):
                src = ci + i - KH
                if 0 <= src < n_in:
                    A[p, src] += wgt * g64[i]
    return A, r0


def _build_weights(log_sigma, log_scale):
    # scalar chain in fp32 to mirror the reference
    scale = _softplus(np.asarray(log_scale, np.float32)).astype(np.float32)
    s_min = np.float32(scale.min())
    sigma_min = np.float32(0.0) if s_min >= 1.0 else np.float32(0.44) * (
        np.float32(1.0) / s_min - np.float32(1.0))
    sigma = np.float32(np.sqrt(sigma_min ** 2
                               + _softplus(np.asarray(log_sigma, np.float32)) ** 2))
    taps = np.arange(-(KH + 1), KH + 1, dtype=np.float32)
    g = np.exp(-0.5 * (taps / sigma) ** 2)
    g = g / g.sum()

    lin = np.linspace(-1.0, 1.0, GRID).astype(np.float32)
    Ay, ry = _axis_weights(lin, g, scale[1], H)  # rows scaled by scale[1] (y)
    Ax, _ = _axis_weights(lin, g, scale[0], W)   # cols scaled by scale[0] (x)
    return Ay, Ax, ry


def _col_window(Amat, n_in):
    """Smallest [start, start + ncw*128) window covering A's nonzero columns."""
    used = np.nonzero(Amat.any(axis=0))[0]
    if len(used) == 0:
        return 0, 1
    lo, hi = int(used[0]), int(used[-1]) + 1
    ncw = min(4, max(1, -(-(hi - lo) // 128)))
    start = max(0, min(lo, n_in - ncw * 128))
    if hi > start + ncw * 128:
        ncw, start = 4, 0
    return start, ncw


def _bf16_split(a32):
    import ml_dtypes
    hi = a32.astype(ml_dtypes.bfloat16)
    lo = (a32 - hi.astype(np.float32)).astype(ml_dtypes.bfloat16)
    return hi, lo


_PROGRAM_CACHE = {}


def _build_program(ncw):
    import concourse.tile as tile
    from concourse import bacc, mybir
    from concourse.bass import ts

    f32 = mybir.dt.float32
    bf16 = mybir.dt.bfloat16
    wsz = ncw * 128

    nc = bacc.Bacc("TRN2", target_bir_lowering=False, debug=False,
                   num_devices=N_CORES)
    # packed gathered rows: (img, 2 chunks x 128 rows, hi/lo, window cols)
    xs = nc.dram_tensor("xs", [NIMG, 2, 128, 2, wsz], bf16, kind="ExternalInput")
    # stage-1 rhs per row chunk; cols interleaved (p, t): 2p = hi, 2p+1 = lo
    ayt = nc.dram_tensor("ayt", [2, 128, 2 * GRID], bf16, kind="ExternalInput")
    axt = nc.dram_tensor("axt", [ncw, 128, GRID], f32, kind="ExternalInput")
    out = nc.dram_tensor("out", [GRID, NIMG, GRID], f32, kind="ExternalOutput")

    with tile.TileContext(nc) as tc:
        with (
            tc.tile_pool(name="const", bufs=1) as const_pool,
            tc.tile_pool(name="xp", bufs=4) as xpool,
            tc.tile_pool(name="ps1", bufs=6, space="PSUM") as psum1,
            tc.tile_pool(name="ps2", bufs=1, space="PSUM") as psum2,
        ):
            aytile = const_pool.tile([128, 2, 2 * GRID], bf16)
            for c in range(2):
                nc.sync.dma_start(out=aytile[:, c, :], in_=ayt[c])
            axtile = const_pool.tile([128, ncw, GRID], f32)
            for c in range(ncw):
                nc.sync.dma_start(out=axtile[:, c, :], in_=axt[c])

            tm = const_pool.tile([128, ncw, NIMG, GRID], f32)
            for img in range(NIMG):
                # one full-width DMA per image: (row-in-chunk, chunk, hi/lo, w)
                xt = xpool.tile([128, 2, 2, wsz], bf16)
                nc.sync.dma_start(
                    out=xt[:], in_=xs[img].rearrange("c p t w -> p c (t w)"))

                for cw in range(ncw):
                    # psum cols interleaved (p, t); evacuated by X-axis reduce
                    ps = psum1.tile([128, GRID, 2], f32)
                    for c in range(2):
                        for t in range(2):
                            nc.tensor.matmul(
                                ps[:],
                                xt[:, c, t, ts(cw, 128)],  # lhsT (K=rows, M=w)
                                aytile[:, c, :],           # rhs  (K=rows, N=(p,t))
                                start=(c == 0 and t == 0),
                                stop=(c == 1 and t == 1),
                            )
                    nc.vector.tensor_reduce(
                        tm[:, cw, img, :], ps[:],
                        axis=mybir.AxisListType.X, op=mybir.AluOpType.add)

            # stage 2: one fp32 matmul per w-chunk for all images
            po = psum2.tile([GRID, NIMG, GRID], f32)
            for cw in range(ncw):
                nc.tensor.matmul(
                    po[:],
                    axtile[:, cw, :],                      # lhsT (K=w, M=q)
                    tm[:, cw, :, :],                       # rhs  (K=w, N=(img,p))
                    start=(cw == 0),
                    stop=(cw == ncw - 1),
                )
            outst = const_pool.tile([GRID, NIMG, GRID], f32)
            nc.vector.tensor_copy(outst[:], po[:])
            nc.sync.dma_start(out=out[:], in_=outst[:])

    nc.compile()
    return nc


def _get_program(ncw):
    if ncw not in _PROGRAM_CACHE:
        _PROGRAM_CACHE[ncw] = _build_program(ncw)
    return _PROGRAM_CACHE[ncw]


def _gather_ay(Ay, ry):
    """Stage-1 rhs chunks: gathered row k = 9*p + j holds Ay[p, ry[p]+j],
    masked so it only feeds output sample p; cols interleaved (p, hi/lo)."""
    ayt64 = np.zeros((2, 128, GRID), np.float64)
    for p in range(GRID):
        for j in range(BAND):
            k = BAND * p + j
            ayt64[k // 128, k % 128, p] = Ay[p, int(ry[p]) + j]
    for p in range(GRID):
        sup = np.nonzero(Ay[p])[0]
        if len(sup) and not (ry[p] <= sup[0] and sup[-1] < ry[p] + BAND):
            raise AssertionError("band does not cover sample support")
    hi, lo = _bf16_split(ayt64.astype(np.float32))
    outw = np.zeros((2, 128, 2 * GRID), hi.dtype)
    outw[:, :, 0::2] = hi
    outw[:, :, 1::2] = lo
    return outw


def _prepare(log_sigma, log_scale):
    Ay, Ax, ry = _build_weights(log_sigma, log_scale)
    w0, ncw = _col_window(Ax, W)
    ayt = _gather_ay(Ay, ry)

    def colchunks(A, start, nchk):
        pad = np.zeros((GRID, max(0, start + nchk * 128 - A.shape[1])))
        Aw = np.concatenate([A, pad], axis=1)[:, start:start + nchk * 128]
        return np.ascontiguousarray(Aw.T.reshape(nchk, 128, GRID)).astype(np.float32)

    axt = colchunks(Ax, w0, ncw)
    return ayt, axt, ry, w0, ncw


def _pack_x(x, ry, w0, ncw):
    """Gather banded rows, crop columns, split bf16 hi/lo, pad to 256 rows.
    Returns (B*C, 2, 128, 2, wsz) bf16."""
    wsz = ncw * 128
    xf = x.reshape(B * C, H, W)
    rows = (np.repeat(np.asarray(ry, np.int64), BAND)
            + np.tile(np.arange(BAND), GRID))        # (225,)
    crop = np.zeros((B * C, NGP, wsz), np.float32)
    lo_w, hi_w = w0, min(W, w0 + wsz)
    crop[:, :NG, :hi_w - lo_w] = xf[:, rows, lo_w:hi_w]
    hi, lo = _bf16_split(crop)
    xp = np.stack([hi, lo], axis=2)                  # (BC, 256, 2, wsz)
    return np.ascontiguousarray(
        xp.reshape(B * C, 2, 128, 2, wsz))


def kernel(x, log_sigma, log_scale):
    from concourse.bass_utils import run_bass_kernel_spmd

    x = np.ascontiguousarray(np.asarray(x, np.float32))
    assert x.shape == (B, C, H, W), x.shape

    ayt, axt, ry, w0, ncw = _prepare(log_sigma, log_scale)
    nc = _get_program(ncw)
    xp = _pack_x(x, ry, w0, ncw)

    in_maps = [
        {"xs": xp[i * NIMG:(i + 1) * NIMG], "ayt": ayt, "axt": axt}
        for i in range(N_CORES)
    ]
    res = run_bass_kernel_spmd(nc, in_maps, core_ids=list(range(N_CORES)))

    out = np.empty((B * C, GRID, GRID), np.float32)
    for i in range(N_CORES):
        # per-core output is (GRID, NIMG, GRID) = (q, img, p)
        out[i * NIMG:(i + 1) * NIMG] = res.results[i]["out"].transpose(1, 2, 0)
    return out.reshape(B, C, GRID, GRID)


# revision 15
# speedup vs baseline: 2.8470x; 1.0690x over previous
"""Trainium2 Bass kernel for nn_AffineAdapter (Gaussian blur + affine grid_sample).

The reference pipeline (separable 8-tap Gaussian blur -> bilinear grid_sample on a
25x25 grid, align_corners=True, zero padding) is linear in x and separable per
axis, so each (b, c) image reduces to   out = Ay @ X @ Ax^T   with Ay, Ax of
shape (25, 512) combining blur taps and bilinear weights.  Output sample row p
only reads the 9 input rows [y0(p)-3, y0(p)+6) ("band") and similarly only a
~362-column window is ever touched, so only 25*9 = 225 rows x 384 cols of each
512x512 image carry information.

Distribution/layout: pure data parallel over B*C = 128 images -> 16 images per
core on 8 NeuronCores.  While sharding, the host packs each image's 225 banded
rows (cropped to the column window) into a dense block, splitting values into
bf16 hi + lo halves (x = hi + lo exactly; products recover full fp32 accuracy
while TensorE runs at bf16 speed with fast weight loads).  Each core's shard is
(16, 256, 2, 384) bf16; the device reads it with one 128-partition DMA per
image.

Per core on-device:
  stage 1:  tmpT[w, p] (per img) = sum_k Xg[k, w] * Ayg[k, p] over the 225
            gathered rows (2 chunks x hi/lo passes; X is the stationary
            operand so the surviving w axis lands on PSUM partitions)
  stage 2:  out[q, (img, p)] = sum_w Ax[q, w] * tmpT[w, (img, p)]  (one fp32
            matmul per w-chunk for ALL images)
Host computes Ay/Ax from the runtime log_sigma/log_scale inputs and transposes
the gathered (25, 16, 25) per-core outputs back to (B, C, 25, 25).
"""

import sys

if "/opt/trn_rl_repo" not in sys.path:
    sys.path.insert(0, "/opt/trn_rl_repo")

import numpy as np

GRID = 25
K = 7
KH = K // 2          # conv padding = 3
NTAPS = K + 1        # 8 taps (torch arange quirk)
BAND = NTAPS + 1     # 9 rows per output sample row
NG = GRID * BAND     # 225 gathered rows per image
NGP = 256            # padded to 2 x 128 partitions (rows 225.. are zero)
H = W = 512
B, C = 16, 8
N_CORES = 8
NIMG = (B * C) // N_CORES  # images per core


def _softplus(v):
    v = np.asarray(v)
    return np.log1p(np.exp(-np.abs(v))) + np.maximum(v, 0.0)


def _axis_weights(lin, g, scale_ax, n_in):
    """(GRID, n_in) float64 weight matrix + per-sample band starts r0 such that
    the support of row p lies in [r0[p], r0[p] + BAND)."""
    nb = n_in - 1  # blurred length (conv with K+1 taps, pad K//2 shrinks by 1)
    coord = ((lin * np.float32(scale_ax) + np.float32(1.0))
             * np.float32(0.5) * np.float32(nb - 1)).astype(np.float32)
    c0 = np.floor(coord)
    w1 = (coord - c0).astype(np.float64)
    w0 = 1.0 - w1
    A = np.zeros((GRID, n_in), np.float64)
    g64 = g.astype(np.float64)
    r0 = np.zeros(GRID, np.int64)
    for p in range(GRID):
        r0[p] = int(min(max(c0[p] - KH, 0), n_in - BAND))
        for a, wgt in ((0, w0[p]), (1, w1[p])):
            cc = float(c0[p]) + a
            if not (0.0 <= cc <= nb - 1):
                continue  # zero padding_mode: out-of-range corner contributes 0
            ci = int(min(max(cc, 0.0), nb - 1))
            # blurred[ci] = sum_i g[i] * x[ci + i - KH]
            for i in range(NTAPS):
                src = ci + i - KH
                if 0 <= src < n_in:
                    A[p, src] += wgt * g64[i]
    return A, r0


def _build_weights(log_sigma, log_scale):
    # scalar chain in fp32 to mirror the reference
    scale = _softplus(np.asarray(log_scale, np.float32)).astype(np.float32)
    s_min = np.float32(scale.min())
    sigma_min = np.float32(0.0) if s_min >= 1.0 else np.float32(0.44) * (
        np.float32(1.0) / s_min - np.float32(1.0))
    sigma = np.float32(np.sqrt(sigma_min ** 2
                               + _softplus(np.asarray(log_sigma, np.float32)) ** 2))
    taps = np.arange(-(KH + 1), KH + 1, dtype=np.float32)
    g = np.exp(-0.5 * (taps / sigma) ** 2)
    g = g / g.sum()

    lin = np.linspace(-1.0, 1.0, GRID).astype(np.float32)
    Ay, ry = _axis_weights(lin, g, scale[1], H)  # rows scaled by scale[1] (y)
    Ax, _ = _axis_weights(lin, g, scale[0], W)   # cols scaled by scale[0] (x)
    return Ay, Ax, ry


def _col_window(Amat, n_in):
    """Smallest [start, start + ncw*128) window covering A's nonzero columns."""
    used = np.nonzero(Amat.any(axis=0))[0]
    if len(used) == 0:
        return 0, 1
    lo, hi = int(used[0]), int(used[-1]) + 1
    ncw = min(4, max(1, -(-(hi - lo) // 128)))
    start = max(0, min(lo, n_in - ncw * 128))
    if hi > start + ncw * 128:
        ncw, start = 4, 0
    return start, ncw


def _bf16_split(a32):
    import ml_dtypes
    hi = a32.astype(ml_dtypes.bfloat16)
    lo = (a32 - hi.astype(np.float32)).astype(ml_dtypes.bfloat16)
    return hi, lo


_PROGRAM_CACHE = {}


def _build_program(ncw):
    import concourse.tile as tile
    from concourse import bacc, mybir
    from concourse.bass import ts

    f32 = mybir.dt.float32
    bf16 = mybir.dt.bfloat16
    wsz = ncw * 128

    nc = bacc.Bacc("TRN2", target_bir_lowering=False, debug=False,
                   num_devices=N_CORES)
    # packed gathered rows: (img, 128 partitions, 2 chunks, hi/lo, window cols)
    # -> per (img, partition) the payload is one contiguous 3072B run
    xs = nc.dram_tensor("xs", [NIMG, 128, 2, 2, wsz], bf16, kind="ExternalInput")
    # stage-1 rhs per row chunk; cols interleaved (p, t): 2p = hi, 2p+1 = lo
    ayt = nc.dram_tensor("ayt", [2, 128, 2 * GRID], bf16, kind="ExternalInput")
    axt = nc.dram_tensor("axt", [ncw, 128, GRID], f32, kind="ExternalInput")
    out = nc.dram_tensor("out", [GRID, NIMG, GRID], f32, kind="ExternalOutput")

    with tile.TileContext(nc) as tc:
        with (
            tc.tile_pool(name="const", bufs=1) as const_pool,
            tc.tile_pool(name="xp", bufs=NIMG) as xpool,
            tc.tile_pool(name="ps1", bufs=6, space="PSUM") as psum1,
            tc.tile_pool(name="ps2", bufs=2, space="PSUM") as psum2,
        ):
            aytile = const_pool.tile([128, 2, 2 * GRID], bf16)
            for c in range(2):
                nc.sync.dma_start(out=aytile[:, c, :], in_=ayt[c])
            axtile = const_pool.tile([128, ncw, GRID], f32)
            for c in range(ncw):
                nc.sync.dma_start(out=axtile[:, c, :], in_=axt[c])

            tm = const_pool.tile([128, ncw, NIMG, GRID], f32)
            for img in range(NIMG):
                # one full-width DMA per image, 3072B per partition line
                xt = xpool.tile([128, 2, 2, wsz], bf16)
                nc.sync.dma_start(out=xt[:], in_=xs[img])

                for cw in range(ncw):
                    # psum cols interleaved (p, t); evacuated by X-axis reduce
                    ps = psum1.tile([128, GRID, 2], f32)
                    for c in range(2):
                        for t in range(2):
                            nc.tensor.matmul(
                                ps[:],
                                xt[:, c, t, ts(cw, 128)],  # lhsT (K=rows, M=w)
                                aytile[:, c, :],           # rhs  (K=rows, N=(p,t))
                                start=(c == 0 and t == 0),
                                stop=(c == 1 and t == 1),
                            )
                    nc.vector.tensor_reduce(
                        tm[:, cw, img, :], ps[:],
                        axis=mybir.AxisListType.X, op=mybir.AluOpType.add)

            # stage 2: per image-half, one fp32 matmul per w-chunk (lets the
            # first half start while the second half's stage 1 still runs)
            outst = const_pool.tile([GRID, NIMG, GRID], f32)
            HALF = NIMG // 2
            for h in range(2):
                sl = slice(h * HALF, (h + 1) * HALF)
                po = psum2.tile([GRID, HALF, GRID], f32)
                for cw in range(ncw):
                    nc.tensor.matmul(
                        po[:],
                        axtile[:, cw, :],                  # lhsT (K=w, M=q)
                        tm[:, cw, sl, :],                  # rhs  (K=w, N=(img,p))
                        start=(cw == 0),
                        stop=(cw == ncw - 1),
                    )
                nc.vector.tensor_copy(outst[:, sl, :], po[:])
            nc.sync.dma_start(out=out[:], in_=outst[:])

    nc.compile()
    return nc


def _get_program(ncw):
    if ncw not in _PROGRAM_CACHE:
        _PROGRAM_CACHE[ncw] = _build_program(ncw)
    return _PROGRAM_CACHE[ncw]


def _gather_ay(Ay, ry):
    """Stage-1 rhs chunks: gathered row k = 9*p + j holds Ay[p, ry[p]+j],
    masked so it only feeds output sample p; cols interleaved (p, hi/lo)."""
    ayt64 = np.zeros((2, 128, GRID), np.float64)
    for p in range(GRID):
        for j in range(BAND):
            k = BAND * p + j
            ayt64[k // 128, k % 128, p] = Ay[p, int(ry[p]) + j]
    for p in range(GRID):
        sup = np.nonzero(Ay[p])[0]
        if len(sup) and not (ry[p] <= sup[0] and sup[-1] < ry[p] + BAND):
            raise AssertionError("band does not cover sample support")
    hi, lo = _bf16_split(ayt64.astype(np.float32))
    outw = np.zeros((2, 128, 2 * GRID), hi.dtype)
    outw[:, :, 0::2] = hi
    outw[:, :, 1::2] = lo
    return outw


def _prepare(log_sigma, log_scale):
    Ay, Ax, ry = _build_weights(log_sigma, log_scale)
    w0, ncw = _col_window(Ax, W)
    ayt = _gather_ay(Ay, ry)

    def colchunks(A, start, nchk):
        pad = np.zeros((GRID, max(0, start + nchk * 128 - A.shape[1])))
        Aw = np.concatenate([A, pad], axis=1)[:, start:start + nchk * 128]
        return np.ascontiguousarray(Aw.T.reshape(nchk, 128, GRID)).astype(np.float32)

    axt = colchunks(Ax, w0, ncw)
    return ayt, axt, ry, w0, ncw


def _pack_x(x, ry, w0, ncw):
    """Gather banded rows, crop columns, split bf16 hi/lo, pad to 256 rows.
    Returns (B*C, 128, 2, 2, wsz) bf16 — partition-major layout so each
    (img, partition) is one contiguous 3072B DMA run."""
    wsz = ncw * 128
    xf = x.reshape(B * C, H, W)
    rows = (np.repeat(np.asarray(ry, np.int64), BAND)
            + np.tile(np.arange(BAND), GRID))        # (225,)
    crop = np.zeros((B * C, NGP, wsz), np.float32)
    lo_w, hi_w = w0, min(W, w0 + wsz)
    crop[:, :NG, :hi_w - lo_w] = xf[:, rows, lo_w:hi_w]
    hi, lo = _bf16_split(crop)
    xp = np.stack([hi, lo], axis=2)                  # (BC, 256, 2, wsz)
    xp = xp.reshape(B * C, 2, 128, 2, wsz)           # (BC, chunk, part, t, w)
    return np.ascontiguousarray(xp.transpose(0, 2, 1, 3, 4))


def kernel(x, log_sigma, log_scale):
    from concourse.bass_utils import run_bass_kernel_spmd

    x = np.ascontiguousarray(np.asarray(x, np.float32))
    assert x.shape == (B, C, H, W), x.shape

    ayt, axt, ry, w0, ncw = _prepare(log_sigma, log_scale)
    nc = _get_program(ncw)
    xp = _pack_x(x, ry, w0, ncw)

    in_maps = [
        {"xs": xp[i * NIMG:(i + 1) * NIMG], "ayt": ayt, "axt": axt}
        for i in range(N_CORES)
    ]
    res = run_bass_kernel_spmd(nc, in_maps, core_ids=list(range(N_CORES)))

    out = np.empty((B * C, GRID, GRID), np.float32)
    for i in range(N_CORES):
        # per-core output is (GRID, NIMG, GRID) = (q, img, p)
        out[i * NIMG:(i + 1) * NIMG] = res.results[i]["out"].transpose(1, 2, 0)
    return out.reshape(B, C, GRID, GRID)


# revision 16
# speedup vs baseline: 3.3589x; 1.1798x over previous
"""Trainium2 Bass kernel for nn_AffineAdapter (Gaussian blur + affine grid_sample).

The reference pipeline (separable 8-tap Gaussian blur -> bilinear grid_sample on a
25x25 grid, align_corners=True, zero padding) is linear in x and separable per
axis, so each (b, c) image reduces to   out = Ay @ X @ Ax^T   with Ay, Ax of
shape (25, 512) combining blur taps and bilinear weights.  Output sample row p
only reads the 9 input rows [y0(p)-3, y0(p)+6) ("band") and similarly only a
~362-column window is ever touched, so only 25*9 = 225 rows x 384 cols of each
512x512 image carry information.

Distribution/layout: pure data parallel over B*C = 128 images -> 16 images per
core on 8 NeuronCores.  While sharding, the host packs each image's 225 banded
rows (cropped to the column window) into a dense block, splitting values into
bf16 hi + lo halves (x = hi + lo exactly; products recover full fp32 accuracy
while TensorE runs at bf16 speed with fast weight loads).  Each core's shard is
(16, 256, 2, 384) bf16; the device reads it with one 128-partition DMA per
image.

Per core on-device:
  stage 1:  tmpT[w, p] (per img) = sum_k Xg[k, w] * Ayg[k, p] over the 225
            gathered rows (2 chunks x hi/lo passes; X is the stationary
            operand so the surviving w axis lands on PSUM partitions)
  stage 2:  out[q, (img, p)] = sum_w Ax[q, w] * tmpT[w, (img, p)]  (one fp32
            matmul per w-chunk for ALL images)
Host computes Ay/Ax from the runtime log_sigma/log_scale inputs and transposes
the gathered (25, 16, 25) per-core outputs back to (B, C, 25, 25).
"""

import sys

if "/opt/trn_rl_repo" not in sys.path:
    sys.path.insert(0, "/opt/trn_rl_repo")

import numpy as np

GRID = 25
K = 7
KH = K // 2          # conv padding = 3
NTAPS = K + 1        # 8 taps (torch arange quirk)
BAND = NTAPS + 1     # 9 rows per output sample row
NG = GRID * BAND     # 225 gathered rows per image
NGP = 256            # padded to 2 x 128 partitions (rows 225.. are zero)
H = W = 512
B, C = 16, 8
N_CORES = 8
NIMG = (B * C) // N_CORES  # images per core


def _softplus(v):
    v = np.asarray(v)
    return np.log1p(np.exp(-np.abs(v))) + np.maximum(v, 0.0)


def _axis_weights(lin, g, scale_ax, n_in):
    """(GRID, n_in) float64 weight matrix + per-sample band starts r0 such that
    the support of row p lies in [r0[p], r0[p] + BAND)."""
    nb = n_in - 1  # blurred length (conv with K+1 taps, pad K//2 shrinks by 1)
    coord = ((lin * np.float32(scale_ax) + np.float32(1.0))
             * np.float32(0.5) * np.float32(nb - 1)).astype(np.float32)
    c0 = np.floor(coord)
    w1 = (coord - c0).astype(np.float64)
    w0 = 1.0 - w1
    A = np.zeros((GRID, n_in), np.float64)
    g64 = g.astype(np.float64)
    r0 = np.zeros(GRID, np.int64)
    for p in range(GRID):
        r0[p] = int(min(max(c0[p] - KH, 0), n_in - BAND))
        for a, wgt in ((0, w0[p]), (1, w1[p])):
            cc = float(c0[p]) + a
            if not (0.0 <= cc <= nb - 1):
                continue  # zero padding_mode: out-of-range corner contributes 0
            ci = int(min(max(cc, 0.0), nb - 1))
            # blurred[ci] = sum_i g[i] * x[ci + i - KH]
            for i in range(NTAPS):
                src = ci + i - KH
                if 0 <= src < n_in:
                    A[p, src] += wgt * g64[i]
    return A, r0


def _build_weights(log_sigma, log_scale):
    # scalar chain in fp32 to mirror the reference
    scale = _softplus(np.asarray(log_scale, np.float32)).astype(np.float32)
    s_min = np.float32(scale.min())
    sigma_min = np.float32(0.0) if s_min >= 1.0 else np.float32(0.44) * (
        np.float32(1.0) / s_min - np.float32(1.0))
    sigma = np.float32(np.sqrt(sigma_min ** 2
                               + _softplus(np.asarray(log_sigma, np.float32)) ** 2))
    taps = np.arange(-(KH + 1), KH + 1, dtype=np.float32)
    g = np.exp(-0.5 * (taps / sigma) ** 2)
    g = g / g.sum()

    lin = np.linspace(-1.0, 1.0, GRID).astype(np.float32)
    Ay, ry = _axis_weights(lin, g, scale[1], H)  # rows scaled by scale[1] (y)
    Ax, _ = _axis_weights(lin, g, scale[0], W)   # cols scaled by scale[0] (x)
    return Ay, Ax, ry


def _col_window(Amat, n_in):
    """Smallest [start, start + ncw*128) window covering A's nonzero columns."""
    used = np.nonzero(Amat.any(axis=0))[0]
    if len(used) == 0:
        return 0, 1
    lo, hi = int(used[0]), int(used[-1]) + 1
    ncw = min(4, max(1, -(-(hi - lo) // 128)))
    start = max(0, min(lo, n_in - ncw * 128))
    if hi > start + ncw * 128:
        ncw, start = 4, 0
    return start, ncw


def _bf16_split(a32):
    import ml_dtypes
    hi = a32.astype(ml_dtypes.bfloat16)
    lo = (a32 - hi.astype(np.float32)).astype(ml_dtypes.bfloat16)
    return hi, lo


_PROGRAM_CACHE = {}


def _build_program(ncw):
    import concourse.tile as tile
    from concourse import bacc, mybir
    from concourse.bass import ts

    f32 = mybir.dt.float32
    bf16 = mybir.dt.bfloat16
    wsz = ncw * 128

    nc = bacc.Bacc("TRN2", target_bir_lowering=False, debug=False,
                   num_devices=N_CORES)
    # packed gathered rows: (img, 128 partitions, 2 chunks, hi/lo, window cols)
    # -> per (img, partition) the payload is one contiguous 3072B run
    xs = nc.dram_tensor("xs", [NIMG, 128, 2, 2, wsz], bf16, kind="ExternalInput")
    # stage-1 rhs per row chunk; cols interleaved (p, t): 2p = hi, 2p+1 = lo
    ayt = nc.dram_tensor("ayt", [2, 128, 2 * GRID], bf16, kind="ExternalInput")
    axt = nc.dram_tensor("axt", [ncw, 128, GRID], f32, kind="ExternalInput")
    out = nc.dram_tensor("out", [GRID, NIMG, GRID], f32, kind="ExternalOutput")

    with tile.TileContext(nc) as tc:
        with (
            tc.tile_pool(name="const", bufs=1) as const_pool,
            tc.tile_pool(name="xp", bufs=NIMG) as xpool,
            tc.tile_pool(name="ps1", bufs=6, space="PSUM") as psum1,
            tc.tile_pool(name="ps2", bufs=2, space="PSUM") as psum2,
        ):
            aytile = const_pool.tile([128, 2, 2 * GRID], bf16)
            for c in range(2):
                nc.sync.dma_start(out=aytile[:, c, :], in_=ayt[c])
            axtile = const_pool.tile([128, ncw, GRID], f32)
            for c in range(ncw):
                nc.sync.dma_start(out=axtile[:, c, :], in_=axt[c])

            tm = const_pool.tile([128, ncw, NIMG, GRID], f32)
            for img in range(NIMG):
                # one full-width DMA per image, 3072B per partition line;
                # alternate the two HWDGE rings so issue isn't sequencer-bound
                xt = xpool.tile([128, 2, 2, wsz], bf16)
                eng = nc.sync if img % 2 == 0 else nc.scalar
                eng.dma_start(out=xt[:], in_=xs[img])

                for cw in range(ncw):
                    # psum cols interleaved (p, t); evacuated by X-axis reduce
                    ps = psum1.tile([128, GRID, 2], f32)
                    for c in range(2):
                        for t in range(2):
                            nc.tensor.matmul(
                                ps[:],
                                xt[:, c, t, ts(cw, 128)],  # lhsT (K=rows, M=w)
                                aytile[:, c, :],           # rhs  (K=rows, N=(p,t))
                                start=(c == 0 and t == 0),
                                stop=(c == 1 and t == 1),
                            )
                    nc.vector.tensor_reduce(
                        tm[:, cw, img, :], ps[:],
                        axis=mybir.AxisListType.X, op=mybir.AluOpType.add)

            # stage 2: per image-half, one fp32 matmul per w-chunk (lets the
            # first half start while the second half's stage 1 still runs)
            outst = const_pool.tile([GRID, NIMG, GRID], f32)
            HALF = NIMG // 2
            for h in range(2):
                sl = slice(h * HALF, (h + 1) * HALF)
                po = psum2.tile([GRID, HALF, GRID], f32)
                for cw in range(ncw):
                    nc.tensor.matmul(
                        po[:],
                        axtile[:, cw, :],                  # lhsT (K=w, M=q)
                        tm[:, cw, sl, :],                  # rhs  (K=w, N=(img,p))
                        start=(cw == 0),
                        stop=(cw == ncw - 1),
                    )
                nc.vector.tensor_copy(outst[:, sl, :], po[:])
            nc.sync.dma_start(out=out[:], in_=outst[:])

    nc.compile()
    return nc


def _get_program(ncw):
    if ncw not in _PROGRAM_CACHE:
        _PROGRAM_CACHE[ncw] = _build_program(ncw)
    return _PROGRAM_CACHE[ncw]


def _gather_ay(Ay, ry):
    """Stage-1 rhs chunks: gathered row k = 9*p + j holds Ay[p, ry[p]+j],
    masked so it only feeds output sample p; cols interleaved (p, hi/lo)."""
    ayt64 = np.zeros((2, 128, GRID), np.float64)
    for p in range(GRID):
        for j in range(BAND):
            k = BAND * p + j
            ayt64[k // 128, k % 128, p] = Ay[p, int(ry[p]) + j]
    for p in range(GRID):
        sup = np.nonzero(Ay[p])[0]
        if len(sup) and not (ry[p] <= sup[0] and sup[-1] < ry[p] + BAND):
            raise AssertionError("band does not cover sample support")
    hi, lo = _bf16_split(ayt64.astype(np.float32))
    outw = np.zeros((2, 128, 2 * GRID), hi.dtype)
    outw[:, :, 0::2] = hi
    outw[:, :, 1::2] = lo
    return outw


def _prepare(log_sigma, log_scale):
    Ay, Ax, ry = _build_weights(log_sigma, log_scale)
    w0, ncw = _col_window(Ax, W)
    ayt = _gather_ay(Ay, ry)

    def colchunks(A, start, nchk):
        pad = np.zeros((GRID, max(0, start + nchk * 128 - A.shape[1])))
        Aw = np.concatenate([A, pad], axis=1)[:, start:start + nchk * 128]
        return np.ascontiguousarray(Aw.T.reshape(nchk, 128, GRID)).astype(np.float32)

    axt = colchunks(Ax, w0, ncw)
    return ayt, axt, ry, w0, ncw


def _pack_x(x, ry, w0, ncw):
    """Gather banded rows, crop columns, split bf16 hi/lo, pad to 256 rows.
    Returns (B*C, 128, 2, 2, wsz) bf16 — partition-major layout so each
    (img, partition) is one contiguous 3072B DMA run."""
    wsz = ncw * 128
    xf = x.reshape(B * C, H, W)
    rows = (np.repeat(np.asarray(ry, np.int64), BAND)
            + np.tile(np.arange(BAND), GRID))        # (225,)
    crop = np.zeros((B * C, NGP, wsz), np.float32)
    lo_w, hi_w = w0, min(W, w0 + wsz)
    crop[:, :NG, :hi_w - lo_w] = xf[:, rows, lo_w:hi_w]
    hi, lo = _bf16_split(crop)
    xp = np.stack([hi, lo], axis=2)                  # (BC, 256, 2, wsz)
    xp = xp.reshape(B * C, 2, 128, 2, wsz)           # (BC, chunk, part, t, w)
    return np.ascontiguousarray(xp.transpose(0, 2, 1, 3, 4))


def kernel(x, log_sigma, log_scale):
    from concourse.bass_utils import run_bass_kernel_spmd

    x = np.ascontiguousarray(np.asarray(x, np.float32))
    assert x.shape == (B, C, H, W), x.shape

    ayt, axt, ry, w0, ncw = _prepare(log_sigma, log_scale)
    nc = _get_program(ncw)
    xp = _pack_x(x, ry, w0, ncw)

    in_maps = [
        {"xs": xp[i * NIMG:(i + 1) * NIMG], "ayt": ayt, "axt": axt}
        for i in range(N_CORES)
    ]
    res = run_bass_kernel_spmd(nc, in_maps, core_ids=list(range(N_CORES)))

    out = np.empty((B * C, GRID, GRID), np.float32)
    for i in range(N_CORES):
        # per-core output is (GRID, NIMG, GRID) = (q, img, p)
        out[i * NIMG:(i + 1) * NIMG] = res.results[i]["out"].transpose(1, 2, 0)
    return out.reshape(B, C, GRID, GRID)
